# revision 49
# baseline (speedup 1.0000x reference)
"""Block-diagonal ZF equalizer (nn_BDEqualizer) as a Trainium2 Bass kernel.

Math: for every resource element (b, s, f) and UE u, solve the 8x8 complex
system H_u x_u = y_u where H_u[i, j] = h[b, 0, 8u+i, u, j, s, f] and
y_u[i] = y[b, 0, 8u+i, s, f].  Output x as [B, 1, 32, S, F, 2] (re/im last).

Strategy (data-parallel over the fft axis, per the sharding hint):
  - 8 cores, each owns a contiguous 128-subcarrier slice of F=1024.
  - Host pre-extracts the block-diagonal channel blocks (pure indexing) and
    ships per-core, plane-major shards hd[j, u, b, s, i, 128] and
    yd[u, b, s, i, 128] (j = matrix column, i = row).
  - On-chip layout: subcarriers on the 128 SBUF partitions, the other RE
    axes (u, b-pair, s) = 112 along the free dim.  Each of the 9 augmented
    matrix columns (8 of H + rhs) is a "plane" of 8 rows; every Gaussian
    elimination step is a full-width elementwise op, with per-RE pivot
    reciprocals.  Unpivoted LU + Jordan back-substitution, complex
    arithmetic as separate re/im tiles.
  - The 112 RE columns are split across TWO elementwise engines that run
    the whole solve independently on disjoint column blocks held in
    separate supertiles: DVE (~1.04 ns/elem fp32) takes ND columns and
    Pool/GPSIMD (~1.98 ns/elem via TensorTensor) takes the rest.  Pool's
    ISA has no reciprocal/divide and no TensorScalarPtr, so pivot
    reciprocals run on DVE for both blocks and factors are computed
    sign-positive (G = +H[i,k]*inv(p)) so Pool only needs plain
    add/sub/mult TensorTensor ops.
  - TensorE transposes move between the DMA-friendly [(u,b,s), f] staging
    layout and the compute layout [f, (u,b,s)]; ScalarE drains PSUM into
    the per-engine supertiles and computes the pivot |p|^2 squares.
  - Loads are plane(j)-major and chunk 1's k=0 elimination runs
    plane-at-a-time so the solve starts as soon as the first planes land;
    chunk 2's load instructions are emitted interleaved with chunk 1's
    solve steps so Act's in-order drain queue never backlogs the chunk
    transition.  Each solution row is stored (TensorE transpose + DMA) as
    soon as its back-substitution step finishes.
"""

import os

import numpy as np

import concourse.bacc as bacc
import concourse.mybir as mybir
from concourse.bass_utils import run_bass_kernel_spmd
from concourse.masks import make_identity
from concourse.tile import TileContext

B, NRX, NR, U, A, S, F = 4, 1, 32, 4, 8, 14, 1024
NCORES = 8
FS = F // NCORES        # 128 subcarriers per core
NB = 2                  # batch entries per chunk
NCH = B // NB           # chunks per core
M = U * NB * S          # 112 RE columns per chunk (u, b, s)
NP = 9                  # augmented planes: 8 matrix columns + rhs
ND = 74                 # RE columns solved on DVE (rest go to Pool/GPSIMD)
F32 = mybir.dt.float32
AL = mybir.AluOpType

LAST_RESULTS = None     # BassKernelResults of the most recent run (for test.py)


def _build():
    nc = bacc.Bacc(trn_type="TRN2")

    hdre = nc.dram_tensor("hd_re", [A, U, B, S, A, FS], F32, kind="ExternalInput")
    hdim = nc.dram_tensor("hd_im", [A, U, B, S, A, FS], F32, kind="ExternalInput")
    ydre = nc.dram_tensor("yd_re", [U, B, S, A, FS], F32, kind="ExternalInput")
    ydim = nc.dram_tensor("yd_im", [U, B, S, A, FS], F32, kind="ExternalInput")
    out = nc.dram_tensor("out", [A, U, B, S, FS, 2], F32, kind="ExternalOutput")

    # (engine, column range) pairs: each engine owns cols [c0, c0+mw) of the
    # M RE columns and a private set of tiles sized to mw.
    def engines():
        return ((nc.vector, 0, ND), (nc.gpsimd, ND, M - ND))

    def off(j, i, mw):
        return (j * A + i) * mw

    with TileContext(nc) as tc:
        with (
            tc.tile_pool(name="consts", bufs=1) as consts,
            tc.tile_pool(name="supers", bufs=2) as supers,
            tc.tile_pool(name="work", bufs=1) as work,
            tc.tile_pool(name="stg", bufs=3) as stg,
            tc.tile_pool(name="stgo", bufs=2) as stgo,
            tc.tile_pool(name="stgx", bufs=4) as stgx,
            tc.tile_pool(name="psin", bufs=5, space="PSUM") as psin,
            tc.tile_pool(name="pso", bufs=2, space="PSUM") as pso_pool,
            tc.tile_pool(name="pswarm", bufs=1, space="PSUM") as pswarm,
        ):
            ident = consts.tile([128, 128], F32)
            make_identity(nc, ident)
            # Warm the PE pstate ramp (full speed needs ~3us of continuous
            # execution) with dummy transposes while the first DMA flies;
            # deprioritized so they never delay a ready real transpose.
            warm = pswarm.tile([128, M], F32)
            with tc.high_priority(offset=-(1 << 20)):
                for _ in range(10):
                    nc.tensor.transpose(warm, ident[:M, :], ident[:M, :M])

            def make_sup():
                sup = {}
                for eng, c0, mw in engines():
                    tag = f"H{c0}"
                    sup[c0] = (
                        supers.tile(
                            [128, (NP + 1) * A * mw], F32,
                            tag=tag + "re", name=tag + "re",
                        ),
                        supers.tile(
                            [128, (NP + 1) * A * mw], F32,
                            tag=tag + "im", name=tag + "im",
                        ),
                    )
                return sup

            def row(T, j, i, mw):
                return T[:, off(j, i, mw) : off(j, i, mw) + mw]

            def rows3(T, j, i0, n, mw):
                base = off(j, i0, mw)
                return T[:, base : base + n * mw].rearrange(
                    "p (r c) -> p r c", r=n
                )

            def load_steps(ci, sup, split_first=False):
                """One closure per (plane, component) load piece; the rhs y
                is plane 8 with an identical [(u,b,s), (i,f)] stage shape.
                With split_first, plane 0's pieces are emitted row-group
                first (both components' row 0-3 before rows 4-7) so the k=0
                pivot chain unblocks as early as possible."""
                b0 = ci * NB

                def dma(j, comp):
                    if j < A:
                        src = (hdre, hdim)[comp][j, :, b0 : b0 + NB]
                    else:
                        src = (ydre, ydim)[comp][:, b0 : b0 + NB]
                    stage = stg.tile([M, A * FS], F32, tag="stage")
                    nc.sync.dma_start(stage, src)
                    return stage

                def tr_drain(stage, j, comp, ig):
                    ps = psin.tile([128, 4 * M], F32, tag="psin")
                    for q in range(4):
                        i = ig * 4 + q
                        nc.tensor.transpose(
                            ps[:, q * M : (q + 1) * M],
                            stage[:, i * FS : (i + 1) * FS],
                            ident[:M, :M],
                        )
                    src4 = ps.rearrange("p (q c) -> p q c", q=4)
                    for eng, c0, mw in engines():
                        base = off(j, ig * 4, mw)
                        dst = sup[c0][comp][
                            :, base : base + 4 * mw
                        ].rearrange("p (q c) -> p q c", q=4)
                        nc.scalar.copy(dst, src4[:, :, c0 : c0 + mw])

                def step(j, comp):
                    stage = dma(j, comp)
                    for ig in range(2):
                        tr_drain(stage, j, comp, ig)

                steps = []
                if split_first:
                    # Plane 0 arrives in row-half DMAs with the k=0 pivot
                    # chain emitted between the two drain waves, so the
                    # solve unblocks as soon as rows 0-3 land.
                    def first(pivot_cb):
                        stages = []
                        for comp in range(2):
                            src = (hdre, hdim)[comp][0, :, b0 : b0 + NB]
                            stage = stg.tile([M, A * FS], F32, tag="stage")
                            nc.sync.dma_start(
                                stage[:, : 4 * FS], src[:, :, :, :4]
                            )
                            stages.append((stage, src))
                        for comp in range(2):
                            tr_drain(stages[comp][0], 0, comp, 0)
                        pivot_cb()
                        for comp in range(2):
                            stage, src = stages[comp]
                            nc.sync.dma_start(
                                stage[:, 4 * FS :], src[:, :, :, 4:]
                            )
                        for comp in range(2):
                            tr_drain(stages[comp][0], 0, comp, 1)
                    steps.append(first)
                    rest = [(j, c) for j in range(1, NP) for c in range(2)]
                else:
                    rest = [(j, c) for j in range(NP) for c in range(2)]
                for j, comp in rest:
                    steps.append(lambda j=j, comp=comp: step(j, comp))
                return steps

            def make_wt():
                wt = {}
                for eng, c0, mw in engines():
                    tg = f"w{c0}"
                    # back-sub q-scratches alias the upper halves of
                    # PAs/PBs (they are free by then)
                    wmax = 4
                    sizes = dict(
                        INV=3 * A * mw, GRe=(A - 1) * mw, GIm=(A - 1) * mw,
                        PAs=wmax * (A - 1) * mw, PBs=wmax * (A - 1) * mw,
                        TD=mw, TU=mw, TR=mw,
                    )
                    wt[c0] = {
                        nm: work.tile([128, sz], F32, tag=tg + nm, name=tg + nm)
                        for nm, sz in sizes.items()
                    }
                return wt

            def inv_pair(w, k, mw, n=None):
                # (ir_k, ii_k) as [128, 2, mw]; broadcast over n rows
                v = w["INV"][:, k * mw : k * mw + 2 * A * mw].rearrange(
                    "p (j c) -> p j c", j=2
                )[:, :, :mw]
                if n is None:
                    return v
                return v[:, :, None, :].broadcast_to([128, 2, n, mw])

            def fwd_pivot(sup, wt, k):
                # Pivot chain first: |p|^2 on Act, +/reciprocal on DVE for
                # BOTH column blocks (Pool's ISA lacks divide/reciprocal).
                # Per-block ops are interleaved so consecutive DVE
                # instructions never form a RAW pair (hides the ~95ns SBUF
                # write-ack latency between dependent small ops).
                for eng, c0, mw in reversed(engines()):
                    w = wt[c0]
                    HRe, HIm = sup[c0]
                    nc.scalar.square(w["TD"], row(HRe, k, k, mw))
                    nc.scalar.square(w["TU"], row(HIm, k, k, mw))
                for eng, c0, mw in reversed(engines()):
                    w = wt[c0]
                    nc.vector.tensor_add(w["TD"], w["TD"], w["TU"])
                for eng, c0, mw in reversed(engines()):
                    w = wt[c0]
                    nc.vector.reciprocal(w["TR"], w["TD"])

            def fwd_irii(sup, wt, k):
                for eng, c0, mw in engines():
                    w = wt[c0]
                    HRe, HIm = sup[c0]
                    a = row(HRe, k, k, mw)
                    b_ = row(HIm, k, k, mw)
                    irk = w["INV"][:, k * mw : (k + 1) * mw]
                    iik = w["INV"][:, (A + k) * mw : (A + k + 1) * mw]
                    eng.tensor_mul(irk, a, w["TR"])
                    eng.tensor_mul(iik, b_, w["TR"])

            def fwd_factors(sup, wt, k):
                n = A - 1 - k
                if n == 0:
                    return
                for eng, c0, mw in engines():
                    w = wt[c0]
                    HRe, HIm = sup[c0]
                    # factors G = +H[i,k] * inv(p), via paired products:
                    #   PA = (a*ir || a*ii),  PB = (b*ir || b*ii)
                    car = rows3(HRe, k, k + 1, n, mw)
                    cai = rows3(HIm, k, k + 1, n, mw)
                    car4 = car[:, None, :, :].broadcast_to([128, 2, n, mw])
                    cai4 = cai[:, None, :, :].broadcast_to([128, 2, n, mw])

                    def sc4(T):
                        return T[:, : 2 * n * mw].rearrange(
                            "p (j r c) -> p j r c", j=2, r=n
                        )

                    def sc_half(T, h):
                        return T[:, h * n * mw : (h + 1) * n * mw]

                    eng.tensor_mul(sc4(w["PAs"]), car4, inv_pair(w, k, mw, n))
                    eng.tensor_mul(sc4(w["PBs"]), cai4, inv_pair(w, k, mw, n))
                    gre = w["GRe"][:, : n * mw]
                    gim = w["GIm"][:, : n * mw]
                    # gre = a*ir + b*ii, gim = b*ir - a*ii  (G = H[i,k]/p)
                    eng.tensor_add(gre, sc_half(w["PAs"], 0), sc_half(w["PBs"], 1))
                    eng.tensor_sub(gim, sc_half(w["PBs"], 0), sc_half(w["PAs"], 1))

            def fwd_updates(sup, wt, k, plane_split):
                # eliminate column k from planes k+1..7 and y, in groups of
                # up to 4 (DVE) / 6 (Pool) planes per instruction (single-
                # plane groups on chunk 1 / k=0 so updates pace with DMA).
                n = A - 1 - k
                if n == 0:
                    return
                for eng, c0, mw in engines():
                    wmax = 4
                    groups = []
                    js = list(range(k + 1, NP))
                    while js:
                        wg = 1 if plane_split else min(wmax, len(js))
                        groups.append((js[0], wg))
                        js = js[wg:]
                    for j0, wg in groups:
                        w = wt[c0]
                        HRe, HIm = sup[c0]

                        def wrows(T):
                            base = off(j0, k + 1, mw)
                            return T[:, base : base + wg * A * mw].rearrange(
                                "p (w c) -> p w c", w=wg
                            )[:, :, : n * mw]

                        def wrow_b(T):
                            base = off(j0, k, mw)
                            v = T[:, base : base + wg * A * mw].rearrange(
                                "p (w c) -> p w c", w=wg
                            )[:, :, :mw]
                            return v[:, :, None, :].broadcast_to(
                                [128, wg, n, mw]
                            )

                        def fw(Ft):
                            v = Ft[:, : n * mw].rearrange(
                                "p (r c) -> p r c", r=n
                            )
                            return v[:, None, :, :].broadcast_to(
                                [128, wg, n, mw]
                            )

                        hr, hi = wrows(HRe), wrows(HIm)
                        Br, Bi = wrow_b(HRe), wrow_b(HIm)
                        grew, gimw = fw(w["GRe"]), fw(w["GIm"])
                        SA4 = w["PAs"][:, : wg * n * mw].rearrange(
                            "p (w r c) -> p w r c", w=wg, r=n
                        )
                        SA3 = w["PAs"][:, : wg * n * mw].rearrange(
                            "p (w c) -> p w c", w=wg
                        )
                        SB4 = w["PBs"][:, : wg * n * mw].rearrange(
                            "p (w r c) -> p w r c", w=wg, r=n
                        )
                        SB3 = w["PBs"][:, : wg * n * mw].rearrange(
                            "p (w c) -> p w c", w=wg
                        )
                        # H[i,j] -= G*B (complex); products regrouped by
                        # factor so consecutive ops never share a RAW
                        # destination (longer dep gaps -> less ack stall)
                        eng.tensor_mul(SA4, grew, Br)
                        eng.tensor_mul(SB4, grew, Bi)
                        eng.tensor_sub(hr, hr, SA3)
                        eng.tensor_sub(hi, hi, SB3)
                        eng.tensor_mul(SA4, gimw, Bi)
                        eng.tensor_mul(SB4, gimw, Br)
                        eng.tensor_add(hr, hr, SA3)
                        eng.tensor_sub(hi, hi, SB3)

            def back_step(sup, wt, ci, k):
                b0 = ci * NB
                # xrow holds x_k full-width (re || im) so a single PE
                # transpose per component can stage the store; 4 bufs keep
                # the slower engine's reads of an older x_k from blocking
                # the faster engine's write of the current one (WAR).
                xrow = stgx.tile([128, 2 * M], F32, tag="xrow", name="xrow")
                for eng, c0, mw in engines():
                    w = wt[c0]
                    HRe, HIm = sup[c0]
                    yr = row(HRe, 8, k, mw)
                    yi = row(HIm, 8, k, mw)
                    # p1 = (yr*ir || yr*ii), p2 = (yi*ir || yi*ii)
                    p1 = w["PAs"][:, : 2 * mw].rearrange("p (j c) -> p j c", j=2)
                    p2 = w["PBs"][:, : 2 * mw].rearrange("p (j c) -> p j c", j=2)
                    yr2 = yr[:, None, :].broadcast_to([128, 2, mw])
                    yi2 = yi[:, None, :].broadcast_to([128, 2, mw])
                    eng.tensor_mul(p1, yr2, inv_pair(w, k, mw))
                    eng.tensor_mul(p2, yi2, inv_pair(w, k, mw))
                    # x = y*conj(p)/|p|^2: xr = yr*ir + yi*ii,
                    #                      xi = yi*ir - yr*ii
                    eng.tensor_add(
                        xrow[:, c0 : c0 + mw],
                        w["PAs"][:, :mw], w["PBs"][:, mw : 2 * mw],
                    )
                    eng.tensor_sub(
                        xrow[:, M + c0 : M + c0 + mw],
                        w["PBs"][:, :mw], w["PAs"][:, mw : 2 * mw],
                    )
                # x_k is final now -- store it while the rest of the back
                # pass still runs on the elementwise engines.
                so = stgo.tile([M, 2 * FS], F32, tag="so")
                so3 = so.rearrange("p (f c) -> p f c", c=2)
                for comp in range(2):
                    po = pso_pool.tile([M, FS], F32, tag="pso")
                    nc.tensor.transpose(
                        po, xrow[:, comp * M : (comp + 1) * M],
                        ident[:128, :128],
                    )
                    nc.scalar.copy(so3[:, :, comp], po)
                nc.sync.dma_start(out[k, :, b0 : b0 + NB], so)
                if k == 0:
                    return
                for eng, c0, mw in engines():
                    w = wt[c0]
                    HRe, HIm = sup[c0]
                    xr_p = xrow[:, c0 : c0 + mw]
                    xi_p = xrow[:, M + c0 : M + c0 + mw]
                    cr = rows3(HRe, k, 0, k, mw)
                    ci_ = rows3(HIm, k, 0, k, mw)
                    xrB = xr_p[:, None, :].broadcast_to([128, k, mw])
                    xiB = xi_p[:, None, :].broadcast_to([128, k, mw])

                    def sc3(T):
                        return T[:, : k * mw].rearrange("p (r c) -> p r c", r=k)

                    def sc3_hi(T):
                        base = 2 * (A - 1) * mw
                        return T[:, base : base + k * mw].rearrange(
                            "p (r c) -> p r c", r=k
                        )

                    qa, qb = sc3(w["PAs"]), sc3(w["PBs"])
                    qc, qd = sc3_hi(w["PAs"]), sc3_hi(w["PBs"])
                    eng.tensor_mul(qa, cr, xrB)
                    eng.tensor_mul(qb, ci_, xiB)
                    eng.tensor_mul(qc, cr, xiB)
                    eng.tensor_mul(qd, ci_, xrB)
                    ytr = rows3(HRe, 8, 0, k, mw)
                    yti = rows3(HIm, 8, 0, k, mw)
                    # y_i -= H[i,k] * x_k
                    eng.tensor_sub(ytr, ytr, qa)
                    eng.tensor_add(ytr, ytr, qb)
                    eng.tensor_sub(yti, yti, qc)
                    eng.tensor_sub(yti, yti, qd)

            # ---- pipelined emission over the two chunks ----
            sup0 = make_sup()
            wt = make_wt()
            steps0 = load_steps(0, sup0, split_first=True)
            steps0[0](lambda: fwd_pivot(sup0, wt, 0))
            for step in steps0[1:]:
                step()
            sup1 = make_sup()
            next_loads = load_steps(1, sup1)

            def solve_chunk(sup, wt, first_chunk):
                for k in range(A):
                    if k > 0 or not first_chunk:
                        fwd_pivot(sup, wt, k)
                    fwd_irii(sup, wt, k)
                    fwd_factors(sup, wt, k)
                    fwd_updates(
                        sup, wt, k,
                        plane_split=(first_chunk and k == 0),
                    )

            li = 0
            solve_chunk(sup0, wt, first_chunk=True)
            # chunk-2 loads are emitted after chunk 1's forward pass
            while li < len(next_loads):
                next_loads[li]()
                li += 1
            for k in range(A - 1, -1, -1):
                back_step(sup0, wt, 0, k)

            solve_chunk(sup1, wt, first_chunk=False)
            for k in range(A - 1, -1, -1):
                back_step(sup1, wt, 1, k)

    nc.finalize()
    return nc


_NC_CACHE = None


def _get_nc():
    global _NC_CACHE
    if _NC_CACHE is None:
        _NC_CACHE = _build()
    return _NC_CACHE


def _prep_core(y_re, y_im, h_re, h_im, c):
    """Host-side shard prep for core c: f-slice + block-diagonal extraction."""
    fsl = slice(c * FS, (c + 1) * FS)
    ue = np.arange(U)
    maps = {}
    for name, h in (("hd_re", h_re), ("hd_im", h_im)):
        h6 = h[:, 0, :, :, :, :, fsl].reshape(B, U, A, U, A, S, FS)
        hd = h6[:, ue, :, ue]              # [u, b, i, j, s, f]
        maps[name] = np.ascontiguousarray(
            hd.transpose(3, 0, 1, 4, 2, 5), dtype=np.float32
        )                                   # [j, u, b, s, i, f]
    for name, y in (("yd_re", y_re), ("yd_im", y_im)):
        y5 = y[:, 0, :, :, fsl].reshape(B, U, A, S, FS)   # [b, u, i, s, f]
        maps[name] = np.ascontiguousarray(
            y5.transpose(1, 0, 3, 2, 4), dtype=np.float32
        )                                   # [u, b, s, i, f]
    return maps


def kernel(y_re, y_im, h_re, h_im, **_ignored):
    global LAST_RESULTS
    y_re = np.asarray(y_re, dtype=np.float32)
    y_im = np.asarray(y_im, dtype=np.float32)
    h_re = np.asarray(h_re, dtype=np.float32)
    h_im = np.asarray(h_im, dtype=np.float32)

    nc = _get_nc()
    in_maps = [_prep_core(y_re, y_im, h_re, h_im, c) for c in range(NCORES)]
    trace = bool(int(os.environ.get("BD_TRACE", "0")))
    res = run_bass_kernel_spmd(
        nc, in_maps, core_ids=list(range(NCORES)), trace=trace
    )
    LAST_RESULTS = res
    outs = []
    for r in res.results:
        o = r["out"]                              # [i, u, b, s, f, c]
        o = o.transpose(2, 1, 0, 3, 4, 5)         # [b, u, i, s, f, c]
        outs.append(o.reshape(B, NR, S, FS, 2))
    full = np.concatenate(outs, axis=3)           # [B, NR, S, F, 2]
    return np.ascontiguousarray(full[:, None])    # [B, 1, NR, S, F, 2]
